# revision 32
# baseline (speedup 1.0000x reference)
"""Trainium2 Bass kernel for a 2-layer GCN + linear classifier (PyG GCNConv style).

Self-contained: hardcodes the 8-core sharding strategy; all graph/index
preprocessing is host-side numpy, all FLOPs on x run on device.

Sharding: nodes are split into 8 contiguous canonical blocks (one per core,
padded to a multiple of 128; slot 0 of each block is a guaranteed-zero pad
row). Per GCN layer each core computes its block's dense transform (bf16
PE matmuls, f32 PSUM), pre-scales rows by dinv, and two AllGathers (split
so gathers can start after the first) materialize the full bf16 node table
in every core's HBM. Each core aggregates its own destinations' in-edges
with gpsimd dma_gather (256B bf16 rows, <=1024 indices per instruction —
ucode descriptor-ring limit — over 4 SWDGE queues) plus DVE/ACT tree
reductions into f32, in two passes split by physical table half so gather
indices fit int16. Destinations are degree-sorted per (core, pass) to
minimize slab padding; a small canonicalization gather restores node order.
Self-loop terms are added as a local elementwise pass (no gather). The
bottleneck is Pool-engine descriptor generation at ~2.6 ns/index.
"""

import sys
import types

import numpy as np


def _setup_env():
    if "/opt/trn_rl_repo" not in sys.path:
        sys.path.insert(0, "/opt/trn_rl_repo")
    if "antenv.axon_hooks" not in sys.modules:
        try:
            from trn_agent_boot.trn_boot import _ntff_profile_via_ctypes

            _hook = _ntff_profile_via_ctypes("/opt/axon/libaxon_pjrt.so")
        except Exception:
            _hook = None
        _mod = types.ModuleType("antenv.axon_hooks")
        _mod.get_axon_ntff_profile_hook = lambda: _hook
        _mod.set_axon_ntff_profile_hook = lambda h: None
        sys.modules["antenv.axon_hooks"] = _mod


_setup_env()

import ml_dtypes  # noqa: E402
from concourse import bacc, bass, mybir, tile  # noqa: E402
import concourse.bass_utils as bass_utils  # noqa: E402
from concourse.bass_utils import run_bass_kernel_spmd  # noqa: E402
from concourse.masks import make_identity  # noqa: E402

bass_utils.upload_artifacts = lambda tmpdir: tmpdir

# --- queue-aware DMASW semaphore lane assignment -----------------------------
# Tile assigns Pool-engine DMA instructions to the 8 DMASW semaphore lanes
# round-robin in *scheduled* order, but each lane gets locked to the SWDGE
# queue of the first instruction using it. With multi-queue dma_gather this
# races; pin each queue to its own lane subset instead.
import concourse.tile_sem_assignment as _tsa  # noqa: E402
from concourse.bass_isa import UserSyncedRemoteDMADescs as _URD  # noqa: E402
from concourse.tile_sem_assignment import DMAInst as _DMAInst  # noqa: E402

_orig_assign_tick = _tsa.TileClockTick._assign_tick


def _queue_aware_assign_tick(self, inst):
    if (
        isinstance(inst, _DMAInst)
        and not isinstance(inst, _URD)
        and inst.engine == mybir.EngineType.Pool
    ):
        q = getattr(inst, "queue_num", 0) or 0
        lanes = max(1, self.swdge_sem_count // NQ)
        rot = self.__dict__.setdefault("_q_lane_rot", {})
        r = rot.get(q, 0)
        self.next_sw_dma_idx = (q * lanes + r) % self.swdge_sem_count
        rot[q] = (r + 1) % lanes
    return _orig_assign_tick(self, inst)


_tsa.TileClockTick._assign_tick = _queue_aware_assign_tick
# -----------------------------------------------------------------------------

import os  # noqa: E402

N_CORES = 8
P = 128
CHUNK = 8   # max gather slabs (of 128 rows) per dma_gather instruction
            # (hard ucode limit: ~1024 idxs per instruction)
NQ = int(os.environ.get("KNQ", "4"))  # SWDGE queues (desc-gen parallelism)
GBUFS = int(os.environ.get("KGBUFS", "6"))
SCAP = int(os.environ.get("KSCAP", "24"))  # max staging slabs per group
TBL16 = os.environ.get("KTBL16", "1") == "1"  # bf16 gather table
NOTREE = os.environ.get("KNOTREE", "") == "1"

dt = mybir.dt
BF16 = ml_dtypes.bfloat16


# ----------------------------------------------------------------------------
# Host-side preprocessing
# ----------------------------------------------------------------------------

def _wrap16(flat: np.ndarray) -> np.ndarray:
    """Lay out an index list in dma_gather's [128, n/16] wrapped format."""
    n = flat.shape[0]
    assert n % 16 == 0
    w = flat.reshape(n // 16, 16).T.astype(np.int16)  # [16, n//16]
    return np.tile(w, (8, 1))  # replicate across the 8 groups of 16 partitions


def _preprocess(x, edge_index, W1, b1, W2, b2, Wfc, bfc):
    N, IN = x.shape
    HID = W1.shape[1]
    CLS = Wfc.shape[1]
    E = edge_index.shape[1]
    assert IN % P == 0 and HID == P

    BLK_RAW = -(-N // N_CORES)            # nodes per core before padding
    BLK = -(-BLK_RAW // P) * P            # padded block size
    assert BLK_RAW + 1 <= BLK, "need pad slots per block"
    NPAD = N_CORES * BLK
    MT = BLK // P
    MTA = MT // 2                         # tiles per block in table half A
    HA = MTA * P                          # rows per block in half A
    HB = BLK - HA
    NROWSA = N_CORES * HA                 # physical half-A table rows
    NROWSB = N_CORES * HB
    assert NROWSA < 32768 and NROWSB < 32768

    src = edge_index[0].astype(np.int64)
    dst = edge_index[1].astype(np.int64)

    deg = np.bincount(dst, minlength=N).astype(np.float64) + 1.0
    dinv = (1.0 / np.sqrt(deg)).astype(np.float32)
    dinv_c = np.zeros(NPAD, dtype=np.float32)
    all_ids = np.arange(N, dtype=np.int64)
    # block-local slot: j=0 reserved as a guaranteed-zero pad row (half A),
    # reals at j in [1, BLK_RAW], remaining pads at the tail (half B).
    canon = (all_ids // BLK_RAW) * BLK + 1 + (all_ids % BLK_RAW)
    dinv_c[canon] = dinv

    def phys(c):
        r = c // BLK
        j = c % BLK
        return np.where(j < HA, r * HA + j, NROWSA + r * HB + (j - HA))

    ZROW_A = 0                              # block 0, j=0
    assert BLK_RAW + 1 < BLK, "need a tail pad slot per block"
    assert BLK_RAW + 1 >= HA, "tail pads must land in half B"
    ZROW_B = int(phys(np.array([BLK_RAW + 1]))[0] - NROWSA)

    # canonical edge list WITHOUT self loops (self term added on-device)
    src_c = (src // BLK_RAW) * BLK + 1 + (src % BLK_RAW)
    dst_c = (dst // BLK_RAW) * BLK + 1 + (dst % BLK_RAW)
    src_p = phys(src_c)

    # per (core, pass) structures
    KA = [[] for _ in range(MT)]  # per-tile K candidates (max over cores below)
    KB = [[] for _ in range(MT)]
    per_core = []
    for r in range(N_CORES):
        lo, hi = r * BLK, (r + 1) * BLK
        m = (dst_c >= lo) & (dst_c < hi)
        s_r = src_p[m]
        d_r = dst_c[m] - lo
        passes = []
        for half in (0, 1):
            pm = (s_r >= NROWSA) if half else (s_r < NROWSA)
            s_p = s_r[pm] - half * NROWSA
            d_p = d_r[pm]
            degp = np.bincount(d_p, minlength=BLK)
            perm = np.argsort(degp, kind="stable")       # perm[pos] = local id
            invperm = np.empty(BLK, dtype=np.int64)
            invperm[perm] = np.arange(BLK)
            sorted_deg = degp[perm]
            Kt = sorted_deg.reshape(MT, P).max(axis=1)
            (KA if half == 0 else KB)[0]  # noqa  (placeholder)
            passes.append(dict(s=s_p, d=d_p, invperm=invperm, Kt=Kt,
                               sorted_deg=sorted_deg))
        per_core.append(passes)

    KAg = np.zeros(MT, dtype=np.int64)
    KBg = np.zeros(MT, dtype=np.int64)
    for r in range(N_CORES):
        KAg = np.maximum(KAg, per_core[r][0]["Kt"])
        KBg = np.maximum(KBg, per_core[r][1]["Kt"])
    WA, WB = int(KAg.sum()), int(KBg.sum())
    offA = np.concatenate([[0], np.cumsum(KAg)[:-1]])
    offB = np.concatenate([[0], np.cumsum(KBg)[:-1]])

    def build_grid(info, Kg, off, zrow):
        sumK = int(Kg.sum())
        grid = np.full((sumK, P), zrow, dtype=np.int64)
        s, d, invperm = info["s"], info["d"], info["invperm"]
        pos = invperm[d]
        order = np.argsort(pos, kind="stable")
        pos_s = pos[order]
        s_s = s[order]
        counts = np.bincount(pos_s, minlength=BLK)
        starts = np.concatenate([[0], np.cumsum(counts)[:-1]])
        k = np.arange(len(pos_s)) - starts[pos_s]
        tile_i = pos_s // P
        lane = pos_s % P
        grid[off[tile_i] + k, lane] = s_s
        return grid

    in_maps = []
    xt_blocks = []
    for r in range(N_CORES):
        lo = r * BLK_RAW
        hi = min(N, (r + 1) * BLK_RAW)
        xb = np.zeros((BLK, IN), dtype=np.float32)
        if hi > lo:
            xb[1 : 1 + hi - lo] = x[lo:hi]
        xt_blocks.append(np.ascontiguousarray(xb.T).astype(BF16))

    b1r = np.tile(np.asarray(b1, np.float32)[None, :], (P, 1))
    b2r = np.tile(np.asarray(b2, np.float32)[None, :], (P, 1))
    bfcr = np.tile(np.asarray(bfc, np.float32)[None, :], (P, 1))
    w1 = np.asarray(W1, np.float32).astype(BF16)
    w2 = np.asarray(W2, np.float32).astype(BF16)
    wfc = np.asarray(Wfc, np.float32).astype(BF16)

    for r in range(N_CORES):
        pa, pb = per_core[r]
        gridA = build_grid(pa, KAg, offA, ZROW_A)
        gridB = build_grid(pb, KBg, offB, ZROW_B)
        dv = dinv_c[r * BLK : (r + 1) * BLK].reshape(MT, P).T.copy()  # [P, MT]
        in_maps.append({
            "xt": xt_blocks[r],
            "w1": w1, "w2": w2, "wfc": wfc,
            "b1r": b1r, "b2r": b2r, "bfcr": bfcr,
            "dinv": np.ascontiguousarray(dv),
            "idxa": np.ascontiguousarray(_wrap16(gridA.reshape(-1))),
            "idxb": np.ascontiguousarray(_wrap16(gridB.reshape(-1))),
            "mapa": np.ascontiguousarray(_wrap16(pa["invperm"])),
            "mapb": np.ascontiguousarray(_wrap16(pb["invperm"])),
        })

    meta = dict(N=N, IN=IN, HID=HID, CLS=CLS, BLK=BLK, BLK_RAW=BLK_RAW,
                NPAD=NPAD, MT=MT, MTA=MTA, NROWSA=NROWSA, NROWSB=NROWSB,
                KA=tuple(int(k) for k in KAg), KB=tuple(int(k) for k in KBg))
    return in_maps, meta


# ----------------------------------------------------------------------------
# Device graph
# ----------------------------------------------------------------------------

def _tree_reduce_into(nc, g, n, out_ap):
    """Sum g[:, :n, :] slabs; final level writes into out_ap."""
    if n == 1:
        nc.any.tensor_copy(out_ap, g[:, 0, :])
        return
    while n > 2:
        if n % 2 == 1:
            nc.any.tensor_add(g[:, 0, :], g[:, 0, :], g[:, n - 1, :])
            n -= 1
            if n == 2:
                break
        h = n // 2
        nc.any.tensor_add(g[:, :h, :], g[:, :h, :], g[:, h : 2 * h, :])
        n = h
    nc.any.tensor_add(out_ap, g[:, 0, :], g[:, 1, :])


def _build(meta, stage="full"):
    IN, HID, CLS = meta["IN"], meta["HID"], meta["CLS"]
    BLK, NPAD, MT = meta["BLK"], meta["NPAD"], meta["MT"]
    MTA = meta["MTA"]
    NROWSA, NROWSB = meta["NROWSA"], meta["NROWSB"]
    KA, KB = meta["KA"], meta["KB"]
    WA, WB = sum(KA), sum(KB)
    KC = IN // P

    tdt = dt.bfloat16 if TBL16 else dt.float32

    nc = bacc.Bacc("TRN2", target_bir_lowering=False, debug=False,
                   num_devices=N_CORES, num_swdge_queues=NQ)

    xt = nc.dram_tensor("xt", [IN, BLK], dt.bfloat16, kind="ExternalInput")
    w1 = nc.dram_tensor("w1", [IN, HID], dt.bfloat16, kind="ExternalInput")
    w2 = nc.dram_tensor("w2", [HID, HID], dt.bfloat16, kind="ExternalInput")
    wfc = nc.dram_tensor("wfc", [HID, CLS], dt.bfloat16, kind="ExternalInput")
    b1r = nc.dram_tensor("b1r", [P, HID], dt.float32, kind="ExternalInput")
    b2r = nc.dram_tensor("b2r", [P, HID], dt.float32, kind="ExternalInput")
    bfcr = nc.dram_tensor("bfcr", [P, CLS], dt.float32, kind="ExternalInput")
    dinv = nc.dram_tensor("dinv", [P, MT], dt.float32, kind="ExternalInput")
    idxa = nc.dram_tensor("idxa", [P, WA * 8], dt.int16, kind="ExternalInput")
    idxb = nc.dram_tensor("idxb", [P, WB * 8], dt.int16, kind="ExternalInput")
    mapa = nc.dram_tensor("mapa", [P, BLK // 16], dt.int16, kind="ExternalInput")
    mapb = nc.dram_tensor("mapb", [P, BLK // 16], dt.int16, kind="ExternalInput")
    out = nc.dram_tensor("out", [BLK, CLS], dt.float32, kind="ExternalOutput")

    with tile.TileContext(nc) as tc:
        with (
            tc.tile_pool(name="const", bufs=1) as cpool,
            tc.tile_pool(name="idx", bufs=1) as ipool,
            tc.tile_pool(name="big", bufs=3) as bigpool,
            tc.tile_pool(name="xload", bufs=3) as xpool,
            tc.tile_pool(name="gbuf", bufs=GBUFS) as gpool,
            tc.tile_pool(name="lhsT", bufs=3) as tpool,
            tc.tile_pool(name="ps", bufs=3, space="PSUM") as pspool,
            tc.tile_pool(name="pst", bufs=2, space="PSUM") as pstpool,
            tc.tile_pool(name="dram", bufs=1, space="DRAM") as dpool,
        ):
            # ---- constants ----
            w1sb = cpool.tile([P, KC, HID], dt.bfloat16, tag="w1")
            nc.sync.dma_start(out=w1sb[:], in_=w1[:].rearrange("(c k) h -> k c h", k=P))
            w2sb = cpool.tile([P, HID], dt.bfloat16, tag="w2")
            nc.sync.dma_start(out=w2sb[:], in_=w2[:])
            wfcsb = cpool.tile([P, CLS], dt.bfloat16, tag="wfc")
            nc.sync.dma_start(out=wfcsb[:], in_=wfc[:])
            b1sb = cpool.tile([P, HID], dt.float32, tag="b1")
            nc.sync.dma_start(out=b1sb[:], in_=b1r[:])
            b2sb = cpool.tile([P, HID], dt.float32, tag="b2")
            nc.sync.dma_start(out=b2sb[:], in_=b2r[:])
            bfcsb = cpool.tile([P, CLS], dt.float32, tag="bfc")
            nc.sync.dma_start(out=bfcsb[:], in_=bfcr[:])
            dvsb = cpool.tile([P, MT], dt.float32, tag="dinv")
            nc.sync.dma_start(out=dvsb[:], in_=dinv[:])
            ident = cpool.tile([P, P], dt.float32, tag="ident")
            make_identity(nc, ident[:])

            idxasb = ipool.tile([P, WA * 8], dt.int16, tag="idxa")
            nc.sync.dma_start(out=idxasb[:], in_=idxa[:])
            idxbsb = ipool.tile([P, WB * 8], dt.int16, tag="idxb")
            nc.sync.dma_start(out=idxbsb[:], in_=idxb[:])
            mapasb = ipool.tile([P, BLK // 16], dt.int16, tag="mapa")
            nc.sync.dma_start(out=mapasb[:], in_=mapa[:])
            mapbsb = ipool.tile([P, BLK // 16], dt.int16, tag="mapb")
            nc.sync.dma_start(out=mapbsb[:], in_=mapb[:])

            self_q = [0]  # rotating SWDGE queue assignment

            def emit_dbg(src_ap):
                nc.sync.dma_start(
                    out=out[:].rearrange("(t p) c -> p t c", p=P), in_=src_ap
                )

            h_prev = None
            for layer in (1, 2):
                # ---- dense transform + pre-scale ----
                hs = bigpool.tile([P, MT, HID], tdt, tag="hs")
                for m in range(MT):
                    ps = pspool.tile([P, HID], dt.float32, tag="mm")
                    if layer == 1:
                        xm = xpool.tile([P, KC, P], dt.bfloat16, tag="x")
                        nc.sync.dma_start(
                            out=xm[:],
                            in_=xt[:].rearrange("(c k) m -> k c m", k=P)[
                                :, :, m * P : (m + 1) * P
                            ],
                        )
                        for c in range(KC):
                            nc.tensor.matmul(
                                ps[:], xm[:, c, :], w1sb[:, c, :],
                                start=(c == 0), stop=(c == KC - 1),
                            )
                    else:
                        pst = pstpool.tile([P, P], dt.float32, tag="tr")
                        nc.tensor.transpose(pst[:], h_prev[:, m, :], ident[:])
                        hT = tpool.tile([P, P], dt.bfloat16, tag="hT")
                        nc.any.tensor_copy(hT[:], pst[:])
                        nc.tensor.matmul(ps[:], hT[:], w2sb[:],
                                         start=True, stop=True)
                    nc.vector.tensor_scalar_mul(hs[:, m, :], ps[:],
                                                dvsb[:, m : m + 1])

                if stage == "mm1":
                    emit_dbg(hs[:, :, :CLS])
                    break
                tables = []
                for hseg, (t0, t1, nrows) in enumerate(
                    ((0, MTA, NROWSA), (MTA, MT, NROWSB))
                ):
                    if t1 == t0:
                        tables.append(None)
                        continue
                    agin = dpool.tile([(t1 - t0) * P, HID], tdt,
                                      tag=f"agin{layer}{hseg}")
                    nc.sync.dma_start(
                        out=agin[:].rearrange("(t p) h -> p t h", p=P),
                        in_=hs[:, t0:t1, :],
                    )
                    tbl = dpool.tile([nrows, HID], tdt,
                                     tag=f"table{layer}{hseg}",
                                     addr_space="Shared")
                    nc.gpsimd.collective_compute(
                        "AllGather",
                        mybir.AluOpType.bypass,
                        replica_groups=[list(range(N_CORES))],
                        ins=[agin[:].opt()],
                        outs=[tbl[:].opt()],
                    )
                    tables.append(tbl)

                if stage == "ag1":
                    nc.sync.dma_start(
                        out=out[:], in_=tables[1][:BLK, :CLS]
                    )
                    break

                # ---- gather + reduce, two passes by source half ----
                pdrams = []
                halves = list(enumerate(((KA, idxasb), (KB, idxbsb))))
                if stage == "gpassA":
                    halves = halves[:1]
                elif stage == "gpassB":
                    halves = halves[1:]
                for half, (Ks, isb) in halves:
                    part = bigpool.tile([P, MT, HID], dt.float32, tag="big")
                    tview = tables[half][:] if tables[half] is not None else None
                    off = 0
                    for t in range(MT):
                        K = Ks[t]
                        if K == 0:
                            nc.vector.memset(part[:, t, :], 0.0)
                            continue
                        g0 = 0
                        first = True
                        while g0 < K:
                            Kg = min(SCAP, K - g0)
                            gt = gpool.tile([P, Kg, HID], tdt, tag="g")
                            g = gt[:]
                            s0 = 0
                            while s0 < Kg:
                                kc = min(CHUNK, Kg - s0)
                                o0 = off + g0 + s0
                                nc.gpsimd.dma_gather(
                                    out_ap=g[:, s0 : s0 + kc, :],
                                    in_ap=tview,
                                    idxs_ap=isb[:, o0 * 8 : (o0 + kc) * 8],
                                    num_idxs=kc * P,
                                    num_idxs_reg=kc * P,
                                    elem_size=HID,
                                    queue_num=self_q[0] % NQ,
                                )
                                self_q[0] += 1
                                s0 += kc
                            if first:
                                _tree_reduce_into(nc, g, Kg, part[:, t, :])
                            else:
                                tmp = tpool.tile([P, P], dt.float32, tag="gtmp")
                                _tree_reduce_into(nc, g, Kg, tmp[:, :HID])
                                nc.any.tensor_add(part[:, t, :], part[:, t, :],
                                                  tmp[:, :HID])
                            first = False
                            g0 += Kg
                        off += K
                    pd = dpool.tile([BLK, HID], dt.float32, tag=f"pd{layer}{half}")
                    nc.sync.dma_start(
                        out=pd[:].rearrange("(t p) h -> p t h", p=P), in_=part[:]
                    )
                    pdrams.append(pd)

                if stage in ("gather1", "gpassA", "gpassB"):
                    nc.sync.dma_start(out=out[:], in_=pdrams[0][:, :CLS])
                    break

                # ---- canonicalize + merge + bias/relu ----
                def canon_gather(dst, pd, mapsb):
                    for c0 in range(0, MT, CHUNK):
                        cc = min(CHUNK, MT - c0)
                        nc.gpsimd.dma_gather(
                            out_ap=dst[:, c0 : c0 + cc, :], in_ap=pd[:],
                            idxs_ap=mapsb[:, c0 * 8 : (c0 + cc) * 8],
                            num_idxs=cc * P, num_idxs_reg=cc * P,
                            elem_size=HID, queue_num=self_q[0] % NQ,
                        )
                        self_q[0] += 1

                accA = bigpool.tile([P, MT, HID], dt.float32, tag="big")
                canon_gather(accA, pdrams[0], mapasb)
                accB = bigpool.tile([P, MT, HID], dt.float32, tag="big")
                canon_gather(accB, pdrams[1], mapbsb)
                hnew = bigpool.tile([P, MT, HID], dt.float32, tag="big")
                bsb = b1sb if layer == 1 else b2sb
                # quarter-width post ops so downstream per-tile consumers
                # (transposes / next matmul / fc) can start early
                QCH = max(1, (MT + 6) // 7)
                for c0 in range(0, MT, QCH):
                    c1 = min(MT, c0 + QCH)
                    sl = slice(c0, c1)
                    w = c1 - c0
                    nc.vector.tensor_add(hnew[:, sl, :], accA[:, sl, :],
                                         accB[:, sl, :])
                    nc.vector.tensor_add(hnew[:, sl, :], hnew[:, sl, :],
                                         hs[:, sl, :])  # self-loop term
                    dv3 = dvsb[:, sl].to_broadcast([P, w, HID])
                    nc.vector.tensor_tensor(hnew[:, sl, :], hnew[:, sl, :],
                                            dv3, op=mybir.AluOpType.mult)
                    b3 = bsb[:].rearrange("p (o h) -> p o h", o=1).to_broadcast(
                        [P, w, HID])
                    nc.vector.tensor_tensor(hnew[:, sl, :], hnew[:, sl, :],
                                            b3, op=mybir.AluOpType.add)
                    nc.scalar.activation(hnew[:, sl, :], hnew[:, sl, :],
                                         mybir.ActivationFunctionType.Relu)
                h_prev = hnew
                if stage == "h1" and layer == 1:
                    emit_dbg(hnew[:, :, :CLS])
                    break
                if stage == "h2" and layer == 2:
                    emit_dbg(hnew[:, :, :CLS])
                if stage == "l1only" and layer == 1:
                    pass  # continue to layer 2 normally

            if stage == "full":  # noqa: SIM102
                # ---- classifier ----
                outsb = bigpool.tile([P, MT, CLS], dt.float32, tag="outsb")
                for m in range(MT):
                    pst = pstpool.tile([P, P], dt.float32, tag="tr")
                    nc.tensor.transpose(pst[:], h_prev[:, m, :], ident[:])
                    hT = tpool.tile([P, P], dt.bfloat16, tag="hT")
                    nc.any.tensor_copy(hT[:], pst[:])
                    ps2 = pspool.tile([P, CLS], dt.float32, tag="mm2")
                    nc.tensor.matmul(ps2[:], hT[:], wfcsb[:], start=True, stop=True)
                    nc.vector.tensor_add(outsb[:, m, :], ps2[:], bfcsb[:])
                    if m % 7 == 6 or m == MT - 1:
                        m0 = (m // 7) * 7
                        nc.sync.dma_start(
                            out=out[:].rearrange("(t p) c -> p t c", p=P)[
                                :, m0 : m + 1, :
                            ],
                            in_=outsb[:, m0 : m + 1, :],
                        )

    nc.compile()
    return nc


# ----------------------------------------------------------------------------
# Entry point
# ----------------------------------------------------------------------------

_CACHE = {}


def _get_graph(meta):
    key = (meta["IN"], meta["HID"], meta["CLS"], meta["BLK"], meta["NPAD"],
           meta["KA"], meta["KB"])
    if key not in _CACHE:
        _CACHE[key] = _build(meta)
    return _CACHE[key]


def kernel(x, edge_index, W1, b1, W2, b2, Wfc, bfc, _want_profile=False,
           _stage="full"):
    x = np.asarray(x, dtype=np.float32)
    in_maps, meta = _preprocess(np.asarray(x), np.asarray(edge_index),
                                np.asarray(W1), np.asarray(b1),
                                np.asarray(W2), np.asarray(b2),
                                np.asarray(Wfc), np.asarray(bfc))
    if _stage != "full":
        nc = _build(meta, stage=_stage)
    else:
        nc = _get_graph(meta)
    res = run_bass_kernel_spmd(nc, in_maps, core_ids=list(range(N_CORES)),
                               trace=_want_profile)
    N, CLS = meta["N"], meta["CLS"]
    BLK_RAW = meta["BLK_RAW"]
    full = np.empty((N, CLS), dtype=np.float32)
    for r in range(N_CORES):
        lo = r * BLK_RAW
        hi = min(N, (r + 1) * BLK_RAW)
        if hi > lo:
            full[lo:hi] = res.results[r]["out"][1 : 1 + hi - lo]
    if _want_profile:
        return full, res
    return full
